# revision 26
# baseline (speedup 1.0000x reference)
"""Distributed k-NN (top-8 smallest L2 distances) on 8 TRN2 NeuronCores.

Strategy (FAISS-style sharded search):
  - base_data [100000, 128] is padded to [100352, 128] and sharded 8 ways
    along the reference axis (12544 refs/core); queries x are replicated.
  - Each core computes scores s[q, r] = 2*x.b - |b|^2 via TensorE matmuls
    (bf16 inputs, fp32 PSUM accumulation); ranking by s is equivalent to
    ranking by -(distance^2) since |x|^2 is constant per query.
  - Local top-8 per query via the VectorE max8 instruction, reading the
    score chunks directly from PSUM (per-chunk top-8, then top-8 of the
    chunk candidates); candidates are converted to -d^2 before exchange.
  - AllGather the 8*[1024,8] local candidates, re-select top-8 of 64, and
    apply d = sqrt(relu(|x|^2 - s)) on the way out.

Pipeline layout: base ingestion streams in groups of 4 row-tiles (512
refs) through two 1-bank PSUM prep tiles: TensorE transposes the group,
ScalarE evicts it to bf16, ScalarE squares the transposed PSUM tile and
TensorE contracts it with a -1s column to produce the -|b|^2 row segment
in place. Scoring/top-k consumes 1536-ref chunks through two 3-bank PSUM
score tiles (TensorE: 3x N=512 x.b matmuls + 3x K=1 rank-1 -|b|^2
accumulations; VectorE max8 reads the chunk straight from PSUM). Scoring
of chunk c only depends on the few ingest groups covering it, so the
whole kernel pipelines end-to-end; steady state is VectorE-bound (max8
streams 1 element/cycle/partition).
"""

import numpy as np

NQ = 1024
D = 128
NREF = 100000
NCORES = 8
RPC = 12544  # refs per core = 98 * 128 (100352 total after padding)
NBT = RPC // 128
K = 8
QT = NQ // 128
CHUNK = 1536  # score chunk (3 PSUM banks)
GT = 4       # base tiles per ingest group (512 refs)
PAD_VAL = 30000.0  # padding rows: huge |b|^2 -> score ~ -9e8, never in top-8

_CACHE = {}


def _build():
    from concourse import bacc, masks, mybir, tile

    F32 = mybir.dt.float32
    BF16 = mybir.dt.bfloat16
    AF = mybir.ActivationFunctionType

    nc = bacc.Bacc("TRN2", target_bir_lowering=False, debug=False, num_devices=NCORES)

    x_ext = nc.dram_tensor("x", [NQ, D], F32, kind="ExternalInput")
    b_ext = nc.dram_tensor("base", [RPC, D], F32, kind="ExternalInput")
    out_ext = nc.dram_tensor("out", [NQ, K], F32, kind="ExternalOutput")

    # ramp-up: small first chunks so scoring starts as soon as the first
    # ingest group lands; then steady-state CHUNK-wide chunks
    chunks = [(0, 512), (512, 1024)]
    off0 = 1536
    while off0 < RPC:
        w = min(CHUNK, RPC - off0)
        chunks.append((off0, w))
        off0 += w
    NCH = len(chunks)

    groups = []
    g0 = 0
    while g0 < NBT:
        n = min(GT, NBT - g0)
        groups.append((g0, n))
        g0 += n

    with tile.TileContext(nc) as tc:
        with (
            tc.tile_pool(name="const", bufs=1) as cpool,
            tc.tile_pool(name="pers", bufs=1) as pers,
            tc.tile_pool(name="dram", bufs=1, space="DRAM") as dpool,
            tc.tile_pool(name="ld", bufs=4) as ld,
            tc.tile_pool(name="sq", bufs=4) as sqp,
            tc.tile_pool(name="small", bufs=4) as smp,
            tc.tile_pool(name="ps", bufs=2, space="PSUM") as ps,
            tc.tile_pool(name="psprep", bufs=2, space="PSUM") as psprep,
        ):
            ident = cpool.tile([128, 128], F32)
            masks.make_identity(nc, ident[:])
            ones_b = cpool.tile([1, 128], BF16)
            nc.gpsimd.memset(ones_b[:], 1.0)
            neg_col = cpool.tile([128, 1], BF16)
            nc.gpsimd.memset(neg_col[:], -1.0)

            bT = pers.tile([128, RPC], BF16)      # base^T (d on partitions)
            xall = pers.tile([128, NQ], F32)      # x, tile t at cols t*128
            xT2 = pers.tile([128, NQ], BF16)      # 2 * x^T
            xnorm = pers.tile([128, QT], F32)     # |x|^2, col per query tile
            nbrow = pers.tile([1, RPC], BF16)     # -|b|^2 as a single row
            cand = pers.tile([128, QT * NCH * K], F32)
            loc = pers.tile([128, QT * K], F32)
            gath = pers.tile([128, QT * NCORES * K], F32)

            cc_in = dpool.tile([NQ, K], F32)
            cc_out = dpool.tile([NCORES * NQ, K], F32)

            # ---- queries: load, transpose, scale by 2, cast bf16 ----
            # two loads so the first transposes start sooner
            for h0, h1 in ((0, GT), (GT, QT)):
                nc.sync.dma_start(
                    out=xall[:, h0 * 128:h1 * 128].rearrange(
                        "p (a d) -> p a d", d=128
                    ),
                    in_=x_ext[h0 * 128:h1 * 128, :].rearrange(
                        "(a p) d -> p a d", p=128
                    ),
                )
            for t0 in range(0, QT, GT):
                px = psprep.tile([128, GT * 128], F32, tag="prep")
                for a in range(GT):
                    t = t0 + a
                    nc.tensor.transpose(
                        px[:, a * 128:(a + 1) * 128],
                        xall[:, t * 128:(t + 1) * 128], ident[:],
                    )
                    if a % 2 == 1:
                        nc.scalar.activation(
                            out=xT2[:, (t - 1) * 128:(t + 1) * 128],
                            in_=px[:, (a - 1) * 128:(a + 1) * 128],
                            func=AF.Copy, scale=2.0,
                        )

            # ---- base ingestion (streamed groups of GT row-tiles) ----
            def ingest(gi):
                g0, ngt = groups[gi]
                off, w = g0 * 128, ngt * 128
                bnat = ld.tile([128, GT * 128], F32, tag="ld")
                for a0 in range(0, ngt, 4):
                    ab = min(a0 + 4, ngt)
                    nc.sync.dma_start(
                        out=bnat[:, a0 * 128:ab * 128].rearrange(
                            "p (a d) -> p a d", d=128
                        ),
                        in_=b_ext[off + a0 * 128:off + ab * 128, :].rearrange(
                            "(a p) d -> p a d", p=128
                        ),
                    )
                pg = psprep.tile([128, GT * 128], F32, tag="prep")
                for a in range(ngt):
                    nc.tensor.transpose(
                        pg[:, a * 128:(a + 1) * 128],
                        bnat[:, a * 128:(a + 1) * 128], ident[:],
                    )
                nc.scalar.activation(
                    out=bT[:, off:off + w], in_=pg[:, :w], func=AF.Copy, scale=1.0,
                )
                # -|b|^2 row segment: square the transposed tile (exact, from
                # f32 PSUM), contract over d with a -1s column on the PE, and
                # evict the [1, w] PSUM row straight into nbrow
                sqT = sqp.tile([128, GT * 128], BF16, tag="sq")
                nc.scalar.activation(
                    out=sqT[:, :w], in_=pg[:, :w], func=AF.Square,
                )
                nc.tensor.matmul(
                    pg[0:1, :w], lhsT=neg_col[:], rhs=sqT[:, :w],
                    start=True, stop=True,
                )
                nc.scalar.activation(
                    out=nbrow[:, off:off + w], in_=pg[0:1, :w], func=AF.Copy,
                )

            ingested = 0
            covered = 0  # refs ingested so far

            # ---- scoring: chunk-outer, query-tile-inner ----
            for ci, (off, w) in enumerate(chunks):
                # ingest groups covering this chunk plus ~1 group of prefetch
                while ingested < len(groups) and covered < min(
                    off + w + GT * 128, RPC
                ):
                    covered += groups[ingested][1] * 128
                    ingest(ingested)
                    ingested += 1
                for t in range(QT):
                    pc = ps.tile([128, CHUNK], F32, tag="ps")
                    nsl = (w + 511) // 512
                    for j in range(nsl):
                        a, b = j * 512, min((j + 1) * 512, w)
                        nc.tensor.matmul(
                            pc[:, a:b],
                            lhsT=xT2[:, t * 128:(t + 1) * 128],
                            rhs=bT[:, off + a:off + b],
                            start=True, stop=False,
                        )
                    for j in range(nsl):
                        a, b = j * 512, min((j + 1) * 512, w)
                        nc.tensor.matmul(
                            pc[:, a:b],
                            lhsT=ones_b[:],
                            rhs=nbrow[:, off + a:off + b],
                            start=False, stop=True,
                        )
                    ck = t * NCH + ci
                    nc.vector.max(cand[:, ck * K:(ck + 1) * K], pc[:, 0:w])

            # ---- |x|^2 (off the critical path; only needed at the end) ----
            for t in range(QT):
                sq = sqp.tile([128, D], F32, tag="sq")
                nc.scalar.activation(
                    out=sq[:], in_=xall[:, t * 128:(t + 1) * 128],
                    func=AF.Square, accum_out=xnorm[:, t:t + 1],
                )

            # ---- local top-8 of the chunk candidates ----
            nxn = pers.tile([128, QT], F32)
            nc.scalar.activation(out=nxn[:], in_=xnorm[:], func=AF.Copy, scale=-1.0)
            locv = pers.tile([128, QT * K], F32)
            for t in range(QT):
                nc.vector.max(
                    loc[:, t * K:(t + 1) * K],
                    cand[:, t * NCH * K:(t + 1) * NCH * K],
                )
                nc.scalar.activation(
                    out=locv[:, t * K:(t + 1) * K], in_=loc[:, t * K:(t + 1) * K],
                    func=AF.Identity, scale=1.0, bias=nxn[:, t:t + 1],
                )
            for h0, h1 in ((0, QT // 2), (QT // 2, QT)):
                nc.sync.dma_start(
                    out=cc_in[h0 * 128:h1 * 128, :].rearrange(
                        "(t p) k -> p t k", p=128
                    ),
                    in_=locv[:, h0 * K:h1 * K].rearrange("p (t k) -> p t k", k=K),
                )

            # ---- exchange + final merge ----
            nc.gpsimd.collective_compute(
                "AllGather",
                mybir.AluOpType.bypass,
                replica_groups=[list(range(NCORES))],
                ins=[cc_in.opt()],
                outs=[cc_out.opt()],
            )

            # gath free layout is (rank, qtile, k) so this is a 3-dim DMA
            nc.sync.dma_start(
                out=gath[:].rearrange("p (a k) -> p a k", k=K),
                in_=cc_out[:].rearrange("(a p) k -> p a k", p=128),
            )
            gview = gath[:].rearrange("p (r t k) -> p r t k", r=NCORES, t=QT)

            m8all = pers.tile([128, QT * K], F32)
            for t in range(QT):
                nc.vector.max(m8all[:, t * K:(t + 1) * K], gview[:, :, t, :])
            d2all = pers.tile([128, QT * K], F32)
            # d^2 = relu(-v): clamp so bf16 rounding on exact duplicates
            # (d ~ 0) can't drive sqrt negative -> NaN
            nc.scalar.activation(out=d2all[:], in_=m8all[:], func=AF.Relu, scale=-1.0)
            resall = pers.tile([128, QT * K], F32)
            nc.scalar.activation(out=resall[:], in_=d2all[:], func=AF.Sqrt)
            nc.sync.dma_start(
                out=out_ext[:].rearrange("(t p) k -> p t k", p=128),
                in_=resall[:].rearrange("p (t k) -> p t k", k=K),
            )

    nc.compile()
    return nc


def _get_nc():
    if "nc" not in _CACHE:
        _CACHE["nc"] = _build()
    return _CACHE["nc"]


def kernel(x, base_data, k):
    from concourse.bass_utils import run_bass_kernel_spmd

    assert int(k) == K
    x = np.ascontiguousarray(np.asarray(x), dtype=np.float32)
    base = np.ascontiguousarray(np.asarray(base_data), dtype=np.float32)
    assert x.shape == (NQ, D) and base.shape == (NREF, D)

    padded = np.full((NCORES * RPC, D), 0.0, dtype=np.float32)
    padded[:NREF] = base
    padded[NREF:, 0] = PAD_VAL
    shards = padded.reshape(NCORES, RPC, D)

    nc = _get_nc()
    in_maps = [{"x": x, "base": np.ascontiguousarray(shards[i])} for i in range(NCORES)]
    res = run_bass_kernel_spmd(nc, in_maps, core_ids=list(range(NCORES)))
    return np.asarray(res.results[0]["out"], dtype=np.float32)


# revision 53
# speedup vs baseline: 1.0580x; 1.0580x over previous
"""Distributed k-NN (top-8 smallest L2 distances) on 8 TRN2 NeuronCores.

Strategy (FAISS-style sharded search):
  - base_data [100000, 128] is padded to [100352, 128] and sharded 8 ways
    along the reference axis (12544 refs/core); queries x are replicated.
  - Each core computes scores s[q, r] = 2*x.b - |b|^2 via TensorE matmuls
    (bf16 inputs, fp32 PSUM accumulation); ranking by s is equivalent to
    ranking by -(distance^2) since |x|^2 is constant per query.
  - Local top-8 per query via the VectorE max8 instruction, reading the
    score chunks directly from PSUM (per-chunk top-8, then top-8 of the
    chunk candidates); candidates are converted to -d^2 before exchange.
  - AllGather the local candidates, re-select top-8 of 64, and apply
    d = sqrt(relu(-v)) on the way out. Scoring runs as two query-tile
    passes (6 tiles, then 2): the first pass's AllGather is issued early
    and hides completely under the second pass's compute, and the first
    pass's gather + final merge + output (including the activation-table
    load for sqrt) hide under the second AllGather, leaving only a 64KB
    exchange plus a short merge for 2 query tiles exposed at the end.

Pipeline layout: base ingestion streams in groups of 4 row-tiles (512
refs): a casting GPSIMD DMA loads the group as bf16 (half the HBM
traffic), TensorE transposes it through a 1-bank bf16 PSUM prep tile,
ScalarE evicts + squares it, GPSIMD sums the squares across partitions
(partition_all_reduce), and ScalarE negates the [1, w] result into the
-|b|^2 row. Scoring/top-k consumes 1536-ref chunks through two 3-bank
PSUM score tiles (TensorE: 3x N=512 x.b matmuls + 3x K=1 rank-1 -|b|^2
accumulations; VectorE max8 reads the chunk straight from PSUM). Scoring
of chunk c only depends on the few ingest groups covering it, so the
whole kernel pipelines end-to-end; steady state is VectorE-bound (max8
streams 1 element/cycle/partition) with >20us of headroom on every other
engine.
"""

import numpy as np

NQ = 1024
D = 128
NREF = 100000
NCORES = 8
RPC = 12544  # refs per core = 98 * 128 (100352 total after padding)
NBT = RPC // 128
K = 8
QT = NQ // 128
CHUNK = 1536  # score chunk (3 PSUM banks)
GT = 4       # base tiles per ingest group (512 refs)
PAD_VAL = 30000.0  # padding rows: huge |b|^2 -> score ~ -9e8, never in top-8

_CACHE = {}


def _build():
    from concourse import bacc, bass_isa, masks, mybir, tile

    F32 = mybir.dt.float32
    BF16 = mybir.dt.bfloat16
    AF = mybir.ActivationFunctionType

    nc = bacc.Bacc("TRN2", target_bir_lowering=False, debug=False, num_devices=NCORES)

    x_ext = nc.dram_tensor("x", [NQ, D], F32, kind="ExternalInput")
    b_ext = nc.dram_tensor("base", [RPC, D], F32, kind="ExternalInput")
    out_ext = nc.dram_tensor("out", [NQ, K], F32, kind="ExternalOutput")

    # pass A: small ramp chunks so scoring starts as soon as the first
    # ingest group lands, then steady-state CHUNK-wide chunks
    chunks_a = [(0, 512), (512, 1024)]
    off0 = 1536
    while off0 < RPC:
        w = min(CHUNK, RPC - off0)
        chunks_a.append((off0, w))
        off0 += w
    # pass B: everything is already ingested, no ramp needed
    chunks_b = []
    off0 = 0
    while off0 < RPC:
        w = min(CHUNK, RPC - off0)
        chunks_b.append((off0, w))
        off0 += w
    NCH = max(len(chunks_a), len(chunks_b))

    groups = []
    g0 = 0
    while g0 < NBT:
        n = min(GT, NBT - g0)
        groups.append((g0, n))
        g0 += n

    with tile.TileContext(nc) as tc:
        with (
            tc.tile_pool(name="const", bufs=1) as cpool,
            tc.tile_pool(name="pers", bufs=1) as pers,
            tc.tile_pool(name="dram", bufs=1, space="DRAM") as dpool,
            tc.tile_pool(name="ld", bufs=4) as ld,
            tc.tile_pool(name="sq", bufs=4) as sqp,
            tc.tile_pool(name="small", bufs=4) as smp,
            tc.tile_pool(name="ps", bufs=2, space="PSUM") as ps,
            tc.tile_pool(name="psprep", bufs=2, space="PSUM") as psprep,
        ):
            ident = cpool.tile([128, 128], F32)
            masks.make_identity(nc, ident[:])
            identb = cpool.tile([128, 128], BF16)
            masks.make_identity(nc, identb[:])
            ones_b = cpool.tile([1, 128], BF16)
            nc.gpsimd.memset(ones_b[:], 1.0)

            bT = pers.tile([128, RPC], BF16)      # base^T (d on partitions)
            xall = pers.tile([128, NQ], F32)      # x, tile t at cols t*128
            xT2 = pers.tile([128, NQ], BF16)      # 2 * x^T
            xnorm = pers.tile([128, QT], F32)     # |x|^2, col per query tile
            nbrow = pers.tile([1, RPC], BF16)     # -|b|^2 as a single row
            cand = pers.tile([128, QT * NCH * K], F32)
            loc = pers.tile([128, QT * K], F32)
            gath = pers.tile([128, QT * NCORES * K], F32)

            HA, HB = 7, 1  # query tiles per exchange batch
            cc_in_a = dpool.tile([HA * 128, K], F32)
            cc_out_a = dpool.tile([NCORES * HA * 128, K], F32, addr_space="Shared")
            cc_in_b = dpool.tile([HB * 128, K], F32)
            cc_out_b = dpool.tile([NCORES * HB * 128, K], F32, addr_space="Shared")

            # ---- queries: load, transpose, scale by 2, cast bf16 ----
            # two loads so the first transposes start sooner
            for h0, h1 in ((0, GT), (GT, QT)):
                nc.sync.dma_start(
                    out=xall[:, h0 * 128:h1 * 128].rearrange(
                        "p (a d) -> p a d", d=128
                    ),
                    in_=x_ext[h0 * 128:h1 * 128, :].rearrange(
                        "(a p) d -> p a d", p=128
                    ),
                )
            def xprep(t0):
                px = psprep.tile([128, GT * 128], F32, tag="prep")
                for a in range(GT):
                    t = t0 + a
                    nc.tensor.transpose(
                        px[:, a * 128:(a + 1) * 128],
                        xall[:, t * 128:(t + 1) * 128], ident[:],
                    )
                    if a % 2 == 1:
                        nc.scalar.activation(
                            out=xT2[:, (t - 1) * 128:(t + 1) * 128],
                            in_=px[:, (a - 1) * 128:(a + 1) * 128],
                            func=AF.Copy, scale=2.0,
                        )
            xprep(0)

            # ---- base ingestion (streamed groups of GT row-tiles) ----
            def ingest(gi):
                g0, ngt = groups[gi]
                off, w = g0 * 128, ngt * 128
                # casting DMA (gpsimd): f32 DRAM -> bf16 SBUF, halves traffic
                bnat = ld.tile([128, GT * 128], BF16, tag="ld")
                for a0 in range(0, ngt, 4):
                    ab = min(a0 + 4, ngt)
                    nc.gpsimd.dma_start(
                        out=bnat[:, a0 * 128:ab * 128].rearrange(
                            "p (a d) -> p a d", d=128
                        ),
                        in_=b_ext[off + a0 * 128:off + ab * 128, :].rearrange(
                            "(a p) d -> p a d", p=128
                        ),
                    )
                pg = psprep.tile([128, GT * 128], BF16, tag="prep")
                for a in range(ngt):
                    nc.tensor.transpose(
                        pg[:, a * 128:(a + 1) * 128],
                        bnat[:, a * 128:(a + 1) * 128], identb[:],
                    )
                nc.scalar.activation(
                    out=bT[:, off:off + w], in_=pg[:, :w], func=AF.Copy, scale=1.0,
                )
                # -|b|^2 row segment: square the transposed tile, sum across
                # partitions on the (otherwise idle) GPSIMD, negate on evict
                sqT = sqp.tile([128, GT * 128], BF16, tag="sq")
                nc.scalar.activation(
                    out=sqT[:, :w], in_=pg[:, :w], func=AF.Square,
                )
                red = sqp.tile([128, GT * 128], F32, tag="red")
                nc.gpsimd.partition_all_reduce(
                    red[:, :w], sqT[:, :w], 128, bass_isa.ReduceOp.add
                )
                nc.scalar.activation(
                    out=nbrow[:, off:off + w], in_=red[0:1, :w], func=AF.Copy,
                    scale=-1.0,
                )

            nxn = pers.tile([128, QT], F32)
            locv = pers.tile([128, QT * K], F32)
            # first two base groups ahead of the remaining query transposes so
            # chunk 0 unblocks as early as possible
            ingest(0)
            ingest(1)
            xprep(GT)
            ingested = 2
            covered = 2 * GT * 128  # refs ingested so far

            # ---- scoring: two query-half passes (chunk-outer within each),
            # so the first half's AllGather hides under the second half ----
            passes = [(0, 7), (7, QT)]
            for tlo, thi in passes:
              chunks = chunks_a if tlo == 0 else chunks_b
              nch = len(chunks)
              for ci, (off, w) in enumerate(chunks):
                # ingest groups covering this chunk plus ~1 group of prefetch
                while ingested < len(groups) and covered < min(
                    off + w + GT * 128, RPC
                ):
                    covered += groups[ingested][1] * 128
                    ingest(ingested)
                    ingested += 1
                for t in range(tlo, thi):
                    pc = ps.tile([128, CHUNK], F32, tag="ps")
                    nsl = (w + 511) // 512
                    for j in range(nsl):
                        a, b = j * 512, min((j + 1) * 512, w)
                        nc.tensor.matmul(
                            pc[:, a:b],
                            lhsT=xT2[:, t * 128:(t + 1) * 128],
                            rhs=bT[:, off + a:off + b],
                            start=True, stop=False,
                        )
                    for j in range(nsl):
                        a, b = j * 512, min((j + 1) * 512, w)
                        nc.tensor.matmul(
                            pc[:, a:b],
                            lhsT=ones_b[:],
                            rhs=nbrow[:, off + a:off + b],
                            start=False, stop=True,
                        )
                    ck = t * NCH + ci
                    nc.vector.max(cand[:, ck * K:(ck + 1) * K], pc[:, 0:w])

              # |x|^2 for this half (cheap; needed before locv below)
              for t in range(tlo, thi):
                sq = sqp.tile([128, D], F32, tag="sq")
                nc.scalar.activation(
                    out=sq[:], in_=xall[:, t * 128:(t + 1) * 128],
                    func=AF.Square, accum_out=xnorm[:, t:t + 1],
                )
              nc.scalar.activation(
                  out=nxn[:, tlo:thi], in_=xnorm[:, tlo:thi],
                  func=AF.Copy, scale=-1.0,
              )
              for t in range(tlo, thi):
                nc.vector.max(
                    loc[:, t * K:(t + 1) * K],
                    cand[:, t * NCH * K:t * NCH * K + nch * K],
                )
                nc.scalar.activation(
                    out=locv[:, t * K:(t + 1) * K], in_=loc[:, t * K:(t + 1) * K],
                    func=AF.Identity, scale=1.0, bias=nxn[:, t:t + 1],
                )
              cc_in = cc_in_a if tlo == 0 else cc_in_b
              cc_out = cc_out_a if tlo == 0 else cc_out_b
              nc.sync.dma_start(
                  out=cc_in[:].rearrange("(t p) k -> p t k", p=128),
                  in_=locv[:, tlo * K:thi * K].rearrange("p (t k) -> p t k", k=K),
              )
              nc.gpsimd.collective_compute(
                  "AllGather",
                  mybir.AluOpType.bypass,
                  replica_groups=[list(range(NCORES))],
                  ins=[cc_in.opt()],
                  outs=[cc_out.opt()],
              )

            # ---- fetch + final merge (gath halves: free layout (r, t, k)) ----
            gath_b = pers.tile([128, NCORES * HB * K], F32)
            for cco, g in (
                (cc_out_a, gath[:, :NCORES * HA * K]),
                (cc_out_b, gath_b[:]),
            ):
                nc.sync.dma_start(
                    out=g.rearrange("p (a k) -> p a k", k=K),
                    in_=cco[:].rearrange("(a p) k -> p a k", p=128),
                )

            gv_a = gath[:, :NCORES * HA * K].rearrange(
                "p (r t k) -> p r t k", r=NCORES, t=HA
            )
            gv_b = gath_b[:].rearrange("p (r t k) -> p r t k", r=NCORES, t=HB)

            m8all = pers.tile([128, QT * K], F32)
            for t in range(QT):
                gv, ti = (gv_a, t) if t < 7 else (gv_b, t - 7)
                nc.vector.max(m8all[:, t * K:(t + 1) * K], gv[:, :, ti, :])
            d2all = pers.tile([128, QT * K], F32)
            # d^2 = relu(-v): clamp so bf16 rounding on exact duplicates
            # (d ~ 0) can't drive sqrt negative -> NaN
            nc.scalar.activation(out=d2all[:], in_=m8all[:], func=AF.Relu, scale=-1.0)
            resall = pers.tile([128, QT * K], F32)
            nc.scalar.activation(out=resall[:], in_=d2all[:], func=AF.Sqrt)
            nc.sync.dma_start(
                out=out_ext[:].rearrange("(t p) k -> p t k", p=128),
                in_=resall[:].rearrange("p (t k) -> p t k", k=K),
            )

    nc.compile()
    return nc


def _get_nc():
    if "nc" not in _CACHE:
        _CACHE["nc"] = _build()
    return _CACHE["nc"]


def kernel(x, base_data, k):
    from concourse.bass_utils import run_bass_kernel_spmd

    assert int(k) == K
    x = np.ascontiguousarray(np.asarray(x), dtype=np.float32)
    base = np.ascontiguousarray(np.asarray(base_data), dtype=np.float32)
    assert x.shape == (NQ, D) and base.shape == (NREF, D)

    padded = np.full((NCORES * RPC, D), 0.0, dtype=np.float32)
    padded[:NREF] = base
    padded[NREF:, 0] = PAD_VAL
    shards = padded.reshape(NCORES, RPC, D)

    nc = _get_nc()
    in_maps = [{"x": x, "base": np.ascontiguousarray(shards[i])} for i in range(NCORES)]
    res = run_bass_kernel_spmd(nc, in_maps, core_ids=list(range(NCORES)))
    return np.asarray(res.results[0]["out"], dtype=np.float32)


# revision 66
# speedup vs baseline: 1.0580x; 1.0000x over previous
"""Distributed k-NN (top-8 smallest L2 distances) on 8 TRN2 NeuronCores.

Strategy (FAISS-style sharded search):
  - base_data [100000, 128] is padded to [100352, 128] and sharded 8 ways
    along the reference axis (12544 refs/core); queries x are replicated.
  - Each core computes scores s[q, r] = 2*x.b - |b|^2 via TensorE matmuls
    (bf16 inputs, fp32 PSUM accumulation); ranking by s is equivalent to
    ranking by -(distance^2) since |x|^2 is constant per query.
  - Local top-8 per query via the VectorE max8 instruction, reading the
    score chunks directly from PSUM (per-chunk top-8, then top-8 of the
    chunk candidates); candidates are converted to -d^2 before exchange.
  - AllGather the local candidates, re-select top-8 of 64, and apply
    d = sqrt(relu(-v)) on the way out. Scoring runs as two query-tile
    passes (6 tiles, then 2): the first pass's AllGather is issued early
    and hides completely under the second pass's compute, and the first
    pass's gather + final merge + output (including the activation-table
    load for sqrt) hide under the second AllGather, leaving only a 64KB
    exchange plus a short merge for 2 query tiles exposed at the end.

Pipeline layout: base ingestion streams in groups of 4 row-tiles (512
refs): a casting GPSIMD DMA loads the group as bf16 (half the HBM
traffic), TensorE transposes it through a 1-bank bf16 PSUM prep tile,
ScalarE evicts + squares it, GPSIMD sums the squares across partitions
(partition_all_reduce), and ScalarE negates the [1, w] result into the
-|b|^2 row. Scoring/top-k consumes 1536-ref chunks through two 3-bank
PSUM score tiles (TensorE: 3x N=512 x.b matmuls + 3x K=1 rank-1 -|b|^2
accumulations; VectorE max8 reads the chunk straight from PSUM). Scoring
of chunk c only depends on the few ingest groups covering it, so the
whole kernel pipelines end-to-end; steady state is VectorE-bound (max8
streams 1 element/cycle/partition) with >20us of headroom on every other
engine.
"""

import numpy as np

NQ = 1024
D = 128
NREF = 100000
NCORES = 8
RPC = 12544  # refs per core = 98 * 128 (100352 total after padding)
NBT = RPC // 128
K = 8
QT = NQ // 128
CHUNK = 1536  # score chunk (3 PSUM banks)
GT = 4       # base tiles per ingest group (512 refs)
PAD_VAL = 30000.0  # padding rows: huge |b|^2 -> score ~ -9e8, never in top-8

_CACHE = {}


def _build():
    from concourse import bacc, bass_isa, masks, mybir, tile

    F32 = mybir.dt.float32
    BF16 = mybir.dt.bfloat16
    AF = mybir.ActivationFunctionType

    nc = bacc.Bacc("TRN2", target_bir_lowering=False, debug=False, num_devices=NCORES)

    x_ext = nc.dram_tensor("x", [NQ, D], F32, kind="ExternalInput")
    b_ext = nc.dram_tensor("base", [RPC, D], F32, kind="ExternalInput")
    out_ext = nc.dram_tensor("out", [NQ, K], F32, kind="ExternalOutput")

    # pass A: small ramp chunks so scoring starts as soon as the first
    # ingest group lands, then steady-state CHUNK-wide chunks
    chunks_a = [(0, 512), (512, 1024)]
    off0 = 1536
    while off0 < RPC:
        w = min(CHUNK, RPC - off0)
        chunks_a.append((off0, w))
        off0 += w
    # pass B: everything is already ingested, no ramp needed
    chunks_b = []
    off0 = 0
    while off0 < RPC:
        w = min(CHUNK, RPC - off0)
        chunks_b.append((off0, w))
        off0 += w
    NCH = max(len(chunks_a), len(chunks_b))

    groups = []
    g0 = 0
    while g0 < NBT:
        n = min(GT, NBT - g0)
        groups.append((g0, n))
        g0 += n

    with tile.TileContext(nc) as tc:
        with (
            tc.tile_pool(name="const", bufs=1) as cpool,
            tc.tile_pool(name="pers", bufs=1) as pers,
            tc.tile_pool(name="dram", bufs=1, space="DRAM") as dpool,
            tc.tile_pool(name="ld", bufs=4) as ld,
            tc.tile_pool(name="sq", bufs=4) as sqp,
            tc.tile_pool(name="ps", bufs=2, space="PSUM") as ps,
            tc.tile_pool(name="psprep", bufs=2, space="PSUM") as psprep,
        ):
            ident = cpool.tile([128, 128], F32)
            masks.make_identity(nc, ident[:])
            identb = cpool.tile([128, 128], BF16)
            masks.make_identity(nc, identb[:])
            ones_b = cpool.tile([1, 128], BF16)
            nc.gpsimd.memset(ones_b[:], 1.0)

            bT = pers.tile([128, RPC], BF16)      # base^T (d on partitions)
            xall = pers.tile([128, NQ], F32)      # x, tile t at cols t*128
            xT2 = pers.tile([128, NQ], BF16)      # 2 * x^T
            xnorm = pers.tile([128, QT], F32)     # |x|^2, col per query tile
            nbrow = pers.tile([1, RPC], BF16)     # -|b|^2 as a single row
            cand = pers.tile([128, QT * NCH * K], F32)
            loc = pers.tile([128, QT * K], F32)
            gath = pers.tile([128, QT * NCORES * K], F32)

            HA, HB = 6, 2  # query tiles per exchange batch
            cc_in_a = dpool.tile([HA * 128, K], F32)
            cc_out_a = dpool.tile([NCORES * HA * 128, K], F32, addr_space="Shared")
            cc_in_b = dpool.tile([HB * 128, K], F32)
            cc_out_b = dpool.tile([NCORES * HB * 128, K], F32, addr_space="Shared")

            # ---- queries: load, transpose, scale by 2, cast bf16 ----
            # two loads so the first transposes start sooner
            for h0, h1 in ((0, GT), (GT, QT)):
                nc.sync.dma_start(
                    out=xall[:, h0 * 128:h1 * 128].rearrange(
                        "p (a d) -> p a d", d=128
                    ),
                    in_=x_ext[h0 * 128:h1 * 128, :].rearrange(
                        "(a p) d -> p a d", p=128
                    ),
                )
            def xprep(t0):
                px = psprep.tile([128, GT * 128], F32, tag="prep")
                for a in range(GT):
                    t = t0 + a
                    nc.tensor.transpose(
                        px[:, a * 128:(a + 1) * 128],
                        xall[:, t * 128:(t + 1) * 128], ident[:],
                    )
                    if a % 2 == 1:
                        nc.scalar.activation(
                            out=xT2[:, (t - 1) * 128:(t + 1) * 128],
                            in_=px[:, (a - 1) * 128:(a + 1) * 128],
                            func=AF.Copy, scale=2.0,
                        )
            xprep(0)

            # ---- base ingestion (streamed groups of GT row-tiles) ----
            def ingest(gi):
                g0, ngt = groups[gi]
                off, w = g0 * 128, ngt * 128
                # casting DMA (gpsimd): f32 DRAM -> bf16 SBUF, halves traffic
                bnat = ld.tile([128, GT * 128], BF16, tag="ld")
                for a0 in range(0, ngt, 4):
                    ab = min(a0 + 4, ngt)
                    nc.gpsimd.dma_start(
                        out=bnat[:, a0 * 128:ab * 128].rearrange(
                            "p (a d) -> p a d", d=128
                        ),
                        in_=b_ext[off + a0 * 128:off + ab * 128, :].rearrange(
                            "(a p) d -> p a d", p=128
                        ),
                    )
                pg = psprep.tile([128, GT * 128], BF16, tag="prep")
                for a in range(ngt):
                    nc.tensor.transpose(
                        pg[:, a * 128:(a + 1) * 128],
                        bnat[:, a * 128:(a + 1) * 128], identb[:],
                    )
                nc.scalar.activation(
                    out=bT[:, off:off + w], in_=pg[:, :w], func=AF.Copy, scale=1.0,
                )
                # -|b|^2 row segment: square the transposed tile, sum across
                # partitions on the (otherwise idle) GPSIMD, negate on evict
                sqT = sqp.tile([128, GT * 128], BF16, tag="sq")
                nc.scalar.activation(
                    out=sqT[:, :w], in_=pg[:, :w], func=AF.Square,
                )
                red = sqp.tile([128, GT * 128], F32, tag="red")
                nc.gpsimd.partition_all_reduce(
                    red[:, :w], sqT[:, :w], 128, bass_isa.ReduceOp.add
                )
                nc.scalar.activation(
                    out=nbrow[:, off:off + w], in_=red[0:1, :w], func=AF.Copy,
                    scale=-1.0,
                )

            nxn = pers.tile([128, QT], F32)
            locv = pers.tile([128, QT * K], F32)
            m8all = pers.tile([128, QT * K], F32)
            d2all = pers.tile([128, QT * K], F32)
            resall = pers.tile([128, QT * K], F32)
            # first two base groups ahead of the remaining query transposes so
            # chunk 0 unblocks as early as possible
            ingest(0)
            ingest(1)
            ingest(2)
            xprep(GT)
            ingested = 3
            covered = 3 * GT * 128  # refs ingested so far

            # ---- scoring: two query-half passes (chunk-outer within each),
            # so the first half's AllGather hides under the second half ----
            passes = [(0, 6), (6, QT)]
            for tlo, thi in passes:
              chunks = chunks_a if tlo == 0 else chunks_b
              nch = len(chunks)
              for ci, (off, w) in enumerate(chunks):
                # ingest groups covering this chunk plus ~1 group of prefetch
                while ingested < len(groups) and covered < min(
                    off + w + GT * 128, RPC
                ):
                    covered += groups[ingested][1] * 128
                    ingest(ingested)
                    ingested += 1
                for t in range(tlo, thi):
                    pc = ps.tile([128, CHUNK], F32, tag="ps")
                    nsl = (w + 511) // 512
                    for j in range(nsl):
                        a, b = j * 512, min((j + 1) * 512, w)
                        nc.tensor.matmul(
                            pc[:, a:b],
                            lhsT=xT2[:, t * 128:(t + 1) * 128],
                            rhs=bT[:, off + a:off + b],
                            start=True, stop=False,
                        )
                    for j in range(nsl):
                        a, b = j * 512, min((j + 1) * 512, w)
                        nc.tensor.matmul(
                            pc[:, a:b],
                            lhsT=ones_b[:],
                            rhs=nbrow[:, off + a:off + b],
                            start=False, stop=True,
                        )
                    ck = t * NCH + ci
                    nc.vector.max(cand[:, ck * K:(ck + 1) * K], pc[:, 0:w])

              # |x|^2 for this half (cheap; needed before locv below)
              for t in range(tlo, thi):
                sq = sqp.tile([128, D], F32, tag="sq")
                nc.scalar.activation(
                    out=sq[:], in_=xall[:, t * 128:(t + 1) * 128],
                    func=AF.Square, accum_out=xnorm[:, t:t + 1],
                )
              nc.scalar.activation(
                  out=nxn[:, tlo:thi], in_=xnorm[:, tlo:thi],
                  func=AF.Copy, scale=-1.0,
              )
              for t in range(tlo, thi):
                nc.vector.max(
                    loc[:, t * K:(t + 1) * K],
                    cand[:, t * NCH * K:t * NCH * K + nch * K],
                )
                nc.scalar.activation(
                    out=locv[:, t * K:(t + 1) * K], in_=loc[:, t * K:(t + 1) * K],
                    func=AF.Identity, scale=1.0, bias=nxn[:, t:t + 1],
                )
              cc_in = cc_in_a if tlo == 0 else cc_in_b
              cc_out = cc_out_a if tlo == 0 else cc_out_b
              nc.sync.dma_start(
                  out=cc_in[:].rearrange("(t p) k -> p t k", p=128),
                  in_=locv[:, tlo * K:thi * K].rearrange("p (t k) -> p t k", k=K),
              )
              nc.gpsimd.collective_compute(
                  "AllGather",
                  mybir.AluOpType.bypass,
                  replica_groups=[list(range(NCORES))],
                  ins=[cc_in.opt()],
                  outs=[cc_out.opt()],
              )
              # fetch this half's gathered candidates right away (the DMA is
              # emitted while only this collective exists, so its wait
              # threshold is on this collective alone)
              nt = thi - tlo
              g = gath[:, tlo * NCORES * K:(tlo + nt) * NCORES * K]
              nc.sync.dma_start(
                  out=g.rearrange("p (a k) -> p a k", k=K),
                  in_=cc_out[:].rearrange("(a p) k -> p a k", p=128),
              )

            # ---- final merge + output; the half-A chain overlaps the
            # half-B AllGather ----
            def finish(tlo, thi, cc_out):
                nt = thi - tlo
                g = gath[:, tlo * NCORES * K:(tlo + nt) * NCORES * K]
                gv = g.rearrange("p (r t k) -> p r t k", r=NCORES, t=nt)
                for t in range(tlo, thi):
                    nc.vector.max(
                        m8all[:, t * K:(t + 1) * K], gv[:, :, t - tlo, :]
                    )
                # d^2 = relu(-v): clamp so bf16 rounding on exact duplicates
                # (d ~ 0) can't drive sqrt negative -> NaN
                nc.scalar.activation(
                    out=d2all[:, tlo * K:thi * K], in_=m8all[:, tlo * K:thi * K],
                    func=AF.Relu, scale=-1.0,
                )
                nc.scalar.activation(
                    out=resall[:, tlo * K:thi * K], in_=d2all[:, tlo * K:thi * K],
                    func=AF.Sqrt,
                )
                nc.sync.dma_start(
                    out=out_ext[tlo * 128:thi * 128, :].rearrange(
                        "(t p) k -> p t k", p=128
                    ),
                    in_=resall[:, tlo * K:thi * K].rearrange(
                        "p (t k) -> p t k", k=K
                    ),
                )

            finish(0, HA, cc_out_a)
            finish(HA, QT, cc_out_b)

    nc.compile()
    return nc


def _get_nc():
    if "nc" not in _CACHE:
        _CACHE["nc"] = _build()
    return _CACHE["nc"]


def kernel(x, base_data, k):
    from concourse.bass_utils import run_bass_kernel_spmd

    assert int(k) == K
    x = np.ascontiguousarray(np.asarray(x), dtype=np.float32)
    base = np.ascontiguousarray(np.asarray(base_data), dtype=np.float32)
    assert x.shape == (NQ, D) and base.shape == (NREF, D)

    padded = np.full((NCORES * RPC, D), 0.0, dtype=np.float32)
    padded[:NREF] = base
    padded[NREF:, 0] = PAD_VAL
    shards = padded.reshape(NCORES, RPC, D)

    nc = _get_nc()
    in_maps = [{"x": x, "base": np.ascontiguousarray(shards[i])} for i in range(NCORES)]
    res = run_bass_kernel_spmd(nc, in_maps, core_ids=list(range(NCORES)))
    return np.asarray(res.results[0]["out"], dtype=np.float32)

